# revision 15
# baseline (speedup 1.0000x reference)
"""Depthwise causal conv1d (K=4, dilation=1) on 8 TRN2 NeuronCores.

Reference: x [B=8, T=4096, C=1024] f32, W [4, 1, 1024] f32 (WIO layout),
y[b, t, c] = sum_k W[k, 0, c] * x[b, t - 3 + k, c]  (zero left-pad).

Sharding: pure batch data-parallel — core i computes batch i. On the host we
pre-transpose each batch slice to [C, T+3] (contiguous, causal zero-pad baked
in) so that on-chip the channel dim sits on SBUF partitions (the per-channel
weight becomes a per-partition scalar operand) and the causal time shifts
become free-dim offsets. The device writes y in [C, T] layout; the host
transposes back.

Per-core compute, per channel-group g (8 groups of 128 channels) and time
tile: load x tile [128, tt+3]. Work is split between two fp32-exact paths
to balance engines (VectorE alone would be the bottleneck at ~122us vs the
~94us HBM roofline):
 - DVE path: ScalarE seeds y = x3*W3 (per-partition scale), then 3x
   VectorE scalar_tensor_tensor accumulates the other taps.
 - PE path: per 512-col chunk, 4 accumulating matmuls with diagonal
   [128x128] weight matrices (built on-chip from an identity by ScalarE,
   lazily per group) shift+scale+sum all taps into PSUM; ScalarE evicts.
First/last tiles are tapered small to shrink pipeline fill/drain. Loads go
on the HWDGE ring (nc.sync); stores go on the SWDGE ring (nc.gpsimd) so
compute-gated stores don't head-of-line-block loads — except the last few
tiles' stores, which ride the sync ring once all loads have drained.
"""

from collections import deque

import numpy as np

B, T, C = 8, 4096, 1024
KTAPS = 4
HALO = KTAPS - 1
CG = 128  # channels per partition-group
N_CORES = 8
MM_N = 512  # fp32 moving-operand max free dim / one PSUM bank

# module-level stash so test.py can read profiling info
last_results = None


def _tile_plan(n_groups, t, tt):
    """Return [(g, t0, tt_i, on_pe)] covering [0,t) per group.

    Default full-size plan; for the production shape (8 groups, tt=2048)
    use tapered ends and ~20/64 chunks on the PE path.
    """
    if n_groups == 8 and t == 4096 and tt == 2048:
        per_group = {
            0: [(512, False), (512, False), (1024, False), (2048, True)],
            1: [(2048, True), (2048, False)],
            2: [(2048, False), (2048, True)],
            3: [(2048, True), (2048, False)],
            4: [(2048, False), (2048, False)],
            5: [(2048, True), (2048, False)],
            6: [(2048, False), (2048, False)],
            7: [(2048, False), (1024, False), (512, False), (512, False)],
        }
        plan = []
        for g in range(8):
            t0 = 0
            for tt_i, on_pe in per_group[g]:
                plan.append((g, t0, tt_i, on_pe))
                t0 += tt_i
            assert t0 == t
        return plan
    # generic fallback: uniform tiles, every 3rd on PE
    plan = []
    idx = 0
    for g in range(n_groups):
        for j in range(t // tt):
            plan.append((g, j * tt, tt, idx % 3 == 1))
            idx += 1
    return plan


def _build_program(c=C, t=T, tt=2048, xbufs=8, ybufs=8, psbufs=8, sync_tail=4):
    import concourse.bass as bass  # noqa: F401
    import concourse.tile as tile
    from concourse import bacc, mybir

    nc = bacc.Bacc(
        "TRN2",
        target_bir_lowering=False,
        debug=False,
        enable_asserts=False,
        num_devices=N_CORES,
    )
    n_groups = c // CG
    f32 = mybir.dt.float32
    x_ap = nc.dram_tensor("x_t", [c, t + HALO], f32, kind="ExternalInput").ap()
    w_ap = nc.dram_tensor("w_t", [CG, n_groups * KTAPS], f32, kind="ExternalInput").ap()
    eye_ap = nc.dram_tensor("eye", [CG, CG], f32, kind="ExternalInput").ap()
    out_ap = nc.dram_tensor("out", [c, t], f32, kind="ExternalOutput").ap()

    mult = mybir.AluOpType.mult
    add = mybir.AluOpType.add

    plan = _tile_plan(n_groups, t, tt)
    pe_groups = sorted({g for (g, _, _, on_pe) in plan if on_pe})
    dblk = {g: i * KTAPS * CG for i, g in enumerate(pe_groups)}

    with tile.TileContext(nc) as tc:
        with (
            tc.tile_pool(name="wpool", bufs=1) as wpool,
            tc.tile_pool(name="xpool", bufs=xbufs) as xpool,
            tc.tile_pool(name="ypool", bufs=ybufs) as ypool,
            tc.tile_pool(name="pspool", bufs=psbufs, space="PSUM") as pspool,
        ):
            # tiny dummy ACTIVATE so the ACT function-table load happens
            # during the NEFF preamble instead of on the first seed
            warm = wpool.tile([CG, 1], f32)
            nc.gpsimd.memset(warm[:], 0.0)
            nc.scalar.mul(warm[:], warm[:], 1.0)

            wt = wpool.tile([CG, n_groups * KTAPS], f32)
            nc.sync.dma_start(wt[:], w_ap[:])
            eye = wpool.tile([CG, CG], f32)
            nc.sync.dma_start(eye[:], eye_ap[:])
            wd = None
            if pe_groups:
                wd = wpool.tile([CG, len(pe_groups) * KTAPS * CG], f32)
            diag_built = set()

            tail_stores = deque()  # last tiles' stores, emitted on sync at end
            for ti, (g, t0, tt_i, on_pe) in enumerate(plan):
                r0, r1 = g * CG, (g + 1) * CG
                xt = xpool.tile([CG, tt + HALO], f32, tag="xt")
                xt = xt[:, : tt_i + HALO]
                # x_t is host-padded: column t0 of x_t == time t0 - HALO
                nc.sync.dma_start(xt[:], x_ap[r0:r1, t0 : t0 + tt_i + HALO])
                yt = ypool.tile([CG, tt], f32, tag="yt")
                yt = yt[:, :tt_i]
                if on_pe:
                    if g not in diag_built:
                        # build diag(W[k, g*CG:(g+1)*CG]) on ScalarE, lazily so
                        # early seeds aren't stuck behind all the diag builds
                        diag_built.add(g)
                        for k in range(KTAPS):
                            blk = dblk[g] + k * CG
                            nc.scalar.mul(
                                wd[:, blk : blk + CG],
                                eye[:],
                                wt[:, g * KTAPS + k : g * KTAPS + k + 1],
                            )
                    for c0 in range(0, tt_i, MM_N):
                        ps = pspool.tile([CG, MM_N], f32)
                        for ki, k in enumerate((3, 2, 1, 0)):
                            dcol = dblk[g] + k * CG
                            nc.tensor.matmul(
                                ps[:],
                                wd[:, dcol : dcol + CG],
                                xt[:, c0 + k : c0 + k + MM_N],
                                start=(ki == 0),
                                stop=(ki == KTAPS - 1),
                            )
                        nc.scalar.copy(yt[:, c0 : c0 + MM_N], ps[:])
                else:
                    wcol = g * KTAPS
                    # seed with the last tap on ScalarE (keeps VectorE at 3 ops)
                    nc.scalar.mul(
                        yt[:], xt[:, HALO : HALO + tt_i], wt[:, wcol + 3 : wcol + 4]
                    )
                    for k in (2, 1, 0):
                        nc.vector.scalar_tensor_tensor(
                            yt[:],
                            xt[:, k : k + tt_i],
                            wt[:, wcol + k : wcol + k + 1],
                            yt[:],
                            op0=mult,
                            op1=add,
                        )
                if ti < len(plan) - sync_tail:
                    nc.gpsimd.dma_start(out_ap[r0:r1, t0 : t0 + tt_i], yt[:])
                else:
                    tail_stores.append((out_ap[r0:r1, t0 : t0 + tt_i], yt[:]))
            while tail_stores:
                dst, src = tail_stores.popleft()
                nc.sync.dma_start(dst, src)
    nc.compile()
    return nc


def _prep_weights(W: np.ndarray, c=C) -> np.ndarray:
    # wt[p, g*KTAPS + k] = W[k, 0, g*CG + p]
    n_groups = c // CG
    wk = W.reshape(KTAPS, n_groups, CG)  # [k, g, p]
    return np.ascontiguousarray(wk.transpose(2, 1, 0).reshape(CG, n_groups * KTAPS))


def kernel(x: np.ndarray, W: np.ndarray) -> np.ndarray:
    global last_results
    from concourse.bass_utils import run_bass_kernel_spmd

    x = np.asarray(x, dtype=np.float32)
    W = np.asarray(W, dtype=np.float32)
    assert x.shape == (B, T, C) and W.shape == (KTAPS, 1, C)

    nc = _build_program()
    wt = _prep_weights(W)
    eye = np.eye(CG, dtype=np.float32)
    zpad = np.zeros((C, HALO), dtype=np.float32)
    in_maps = [
        {
            # [C, T+HALO], causal zero left-pad baked in
            "x_t": np.ascontiguousarray(np.concatenate([zpad, x[i].T], axis=1)),
            "w_t": wt,
            "eye": eye,
        }
        for i in range(N_CORES)
    ]
    import os

    # Only trace when the axon NTFF hook is importable; otherwise force
    # tracing off (a stray BASS_TRACE env var would crash bass_utils).
    trace = False
    if os.environ.get("BASS_TRACE") and not os.environ.get("BASS_NEVER_TRACE"):
        try:
            import antenv.axon_hooks  # noqa: F401

            trace = True
        except ImportError:
            os.environ["BASS_NEVER_TRACE"] = "1"
    res = run_bass_kernel_spmd(
        nc, in_maps, core_ids=list(range(N_CORES)), trace=trace
    )
    last_results = res
    y = np.stack([np.asarray(res.results[i]["out"]).T for i in range(N_CORES)])
    return np.ascontiguousarray(y.astype(np.float32))


# revision 21
# speedup vs baseline: 1.0499x; 1.0499x over previous
"""Depthwise causal conv1d (K=4, dilation=1) on 8 TRN2 NeuronCores.

Reference: x [B=8, T=4096, C=1024] f32, W [4, 1, 1024] f32 (WIO layout),
y[b, t, c] = sum_k W[k, 0, c] * x[b, t - 3 + k, c]  (zero left-pad).

Sharding: pure batch data-parallel — core i computes batch i. On the host we
pre-transpose each batch slice to [C, T+3] (contiguous, causal zero-pad baked
in) so that on-chip the channel dim sits on SBUF partitions (the per-channel
weight becomes a per-partition scalar operand) and the causal time shifts
become free-dim offsets. The device writes y in [C, T] layout; the host
transposes back.

Per-core compute, per channel-group g (8 groups of 128 channels) and time
tile: load x tile [128, tt+3]. Work is split between two fp32-exact paths
to balance engines (VectorE alone would be the bottleneck at ~122us vs the
~94us HBM roofline):
 - DVE path: ScalarE seeds y = x3*W3 (per-partition scale), then 3x
   VectorE scalar_tensor_tensor accumulates the other taps.
 - PE path: per 512-col chunk, 4 accumulating matmuls with diagonal
   [128x128] weight matrices (built on-chip from an identity by ScalarE,
   lazily per group) shift+scale+sum all taps into PSUM; ScalarE evicts.
First/last tiles are tapered small to shrink pipeline fill/drain. Loads go
on the HWDGE ring (nc.sync); stores go on the SWDGE ring (nc.gpsimd) so
compute-gated stores don't head-of-line-block loads — except the last few
tiles' stores, which ride the sync ring once all loads have drained.
"""

from collections import deque

import numpy as np

B, T, C = 8, 4096, 1024
KTAPS = 4
HALO = KTAPS - 1
CG = 128  # channels per partition-group
N_CORES = 8
MM_N = 512  # fp32 moving-operand max free dim / one PSUM bank

# module-level stash so test.py can read profiling info
last_results = None


def _tile_plan(n_groups, t, tt, taper=True):
    """Return [(g, t0, tt_i, on_pe)] covering [0,t) per group.

    Default full-size plan; for the production shape (8 groups, tt=2048)
    use tapered ends and ~20/64 chunks on the PE path.
    """
    if taper and n_groups == 8 and t == 4096 and tt == 2048:
        per_group = {
            0: [(512, False), (512, False), (1024, False), (2048, True)],
            1: [(2048, False), (2048, True)],
            2: [(2048, False), (2048, True)],
            3: [(2048, False), (2048, True)],
            4: [(2048, False), (2048, False)],
            5: [(2048, False), (2048, True)],
            6: [(2048, False), (2048, False)],
            7: [(2048, False), (1024, False), (512, False), (512, False)],
        }
        plan = []
        for g in range(8):
            t0 = 0
            for tt_i, on_pe in per_group[g]:
                plan.append((g, t0, tt_i, on_pe))
                t0 += tt_i
            assert t0 == t
        return plan
    # generic fallback: uniform tiles, every 3rd on PE
    plan = []
    idx = 0
    for g in range(n_groups):
        for j in range(t // tt):
            plan.append((g, j * tt, tt, idx % 3 == 1))
            idx += 1
    return plan


def _build_program(
    c=C,
    t=T,
    tt=2048,
    xbufs=8,
    ybufs=8,
    psbufs=8,
    sync_tail=4,
    taper=True,
    seed_prio=30,
):
    import concourse.bass as bass  # noqa: F401
    import concourse.tile as tile
    from concourse import bacc, mybir

    nc = bacc.Bacc(
        "TRN2",
        target_bir_lowering=False,
        debug=False,
        enable_asserts=False,
        num_devices=N_CORES,
    )
    n_groups = c // CG
    f32 = mybir.dt.float32
    x_ap = nc.dram_tensor("x_t", [c, t + HALO], f32, kind="ExternalInput").ap()
    w_ap = nc.dram_tensor("w_t", [CG, n_groups * KTAPS], f32, kind="ExternalInput").ap()
    eye_ap = nc.dram_tensor("eye", [CG, CG], f32, kind="ExternalInput").ap()
    out_ap = nc.dram_tensor("out", [c, t], f32, kind="ExternalOutput").ap()

    mult = mybir.AluOpType.mult
    add = mybir.AluOpType.add

    plan = _tile_plan(n_groups, t, tt, taper=taper)
    pe_groups = sorted({g for (g, _, _, on_pe) in plan if on_pe})
    dblk = {g: i * KTAPS * CG for i, g in enumerate(pe_groups)}

    with tile.TileContext(nc) as tc:
        with (
            tc.tile_pool(name="wpool", bufs=1) as wpool,
            tc.tile_pool(name="xpool", bufs=xbufs) as xpool,
            tc.tile_pool(name="ypool", bufs=ybufs) as ypool,
            tc.tile_pool(name="pspool", bufs=psbufs, space="PSUM") as pspool,
        ):
            # tiny dummy ACTIVATE so the ACT function-table load happens
            # during the NEFF preamble instead of on the first seed
            warm = wpool.tile([CG, 1], f32)
            nc.gpsimd.memset(warm[:], 0.0)
            nc.scalar.mul(warm[:], warm[:], 1.0)

            wt = wpool.tile([CG, n_groups * KTAPS], f32)
            nc.sync.dma_start(wt[:], w_ap[:])
            eye = wpool.tile([CG, CG], f32)
            nc.sync.dma_start(eye[:], eye_ap[:])
            wd = None
            if pe_groups:
                wd = wpool.tile([CG, len(pe_groups) * KTAPS * CG], f32)
            diag_built = set()

            tail_stores = deque()  # last tiles' stores, emitted on sync at end
            for ti, (g, t0, tt_i, on_pe) in enumerate(plan):
                r0, r1 = g * CG, (g + 1) * CG
                xt = xpool.tile([CG, tt + HALO], f32, tag="xt")
                xt = xt[:, : tt_i + HALO]
                # x_t is host-padded: column t0 of x_t == time t0 - HALO
                nc.sync.dma_start(xt[:], x_ap[r0:r1, t0 : t0 + tt_i + HALO])
                yt = ypool.tile([CG, tt], f32, tag="yt")
                yt = yt[:, :tt_i]
                if on_pe:
                    if g not in diag_built:
                        # build diag(W[k, g*CG:(g+1)*CG]) on ScalarE, lazily so
                        # early seeds aren't stuck behind all the diag builds
                        diag_built.add(g)
                        for k in range(KTAPS):
                            blk = dblk[g] + k * CG
                            nc.scalar.mul(
                                wd[:, blk : blk + CG],
                                eye[:],
                                wt[:, g * KTAPS + k : g * KTAPS + k + 1],
                            )
                    for c0 in range(0, tt_i, MM_N):
                        ps = pspool.tile([CG, MM_N], f32)
                        for ki, k in enumerate((3, 2, 1, 0)):
                            dcol = dblk[g] + k * CG
                            nc.tensor.matmul(
                                ps[:],
                                wd[:, dcol : dcol + CG],
                                xt[:, c0 + k : c0 + k + MM_N],
                                start=(ki == 0),
                                stop=(ki == KTAPS - 1),
                            )
                        nc.scalar.copy(yt[:, c0 : c0 + MM_N], ps[:])
                else:
                    wcol = g * KTAPS
                    # seed with the last tap on ScalarE (keeps VectorE at 3 ops);
                    # high priority so seeds never queue behind PE-tile
                    # evictions in ScalarE's in-order stream (starves DVE)
                    if seed_prio:
                        with tc.high_priority(offset=seed_prio):
                            nc.scalar.mul(
                                yt[:],
                                xt[:, HALO : HALO + tt_i],
                                wt[:, wcol + 3 : wcol + 4],
                            )
                    else:
                        nc.scalar.mul(
                            yt[:], xt[:, HALO : HALO + tt_i], wt[:, wcol + 3 : wcol + 4]
                        )
                    for k in (2, 1, 0):
                        nc.vector.scalar_tensor_tensor(
                            yt[:],
                            xt[:, k : k + tt_i],
                            wt[:, wcol + k : wcol + k + 1],
                            yt[:],
                            op0=mult,
                            op1=add,
                        )
                if ti < len(plan) - sync_tail:
                    nc.gpsimd.dma_start(out_ap[r0:r1, t0 : t0 + tt_i], yt[:])
                else:
                    tail_stores.append((out_ap[r0:r1, t0 : t0 + tt_i], yt[:]))
            while tail_stores:
                dst, src = tail_stores.popleft()
                nc.sync.dma_start(dst, src)
    nc.compile()
    return nc


def _prep_weights(W: np.ndarray, c=C) -> np.ndarray:
    # wt[p, g*KTAPS + k] = W[k, 0, g*CG + p]
    n_groups = c // CG
    wk = W.reshape(KTAPS, n_groups, CG)  # [k, g, p]
    return np.ascontiguousarray(wk.transpose(2, 1, 0).reshape(CG, n_groups * KTAPS))


def kernel(x: np.ndarray, W: np.ndarray) -> np.ndarray:
    global last_results
    from concourse.bass_utils import run_bass_kernel_spmd

    x = np.asarray(x, dtype=np.float32)
    W = np.asarray(W, dtype=np.float32)
    assert x.shape == (B, T, C) and W.shape == (KTAPS, 1, C)

    nc = _build_program()
    wt = _prep_weights(W)
    eye = np.eye(CG, dtype=np.float32)
    zpad = np.zeros((C, HALO), dtype=np.float32)
    in_maps = [
        {
            # [C, T+HALO], causal zero left-pad baked in
            "x_t": np.ascontiguousarray(np.concatenate([zpad, x[i].T], axis=1)),
            "w_t": wt,
            "eye": eye,
        }
        for i in range(N_CORES)
    ]
    import os

    # Only trace when the axon NTFF hook is importable; otherwise force
    # tracing off (a stray BASS_TRACE env var would crash bass_utils).
    trace = False
    if os.environ.get("BASS_TRACE") and not os.environ.get("BASS_NEVER_TRACE"):
        try:
            import antenv.axon_hooks  # noqa: F401

            trace = True
        except ImportError:
            os.environ["BASS_NEVER_TRACE"] = "1"
    res = run_bass_kernel_spmd(
        nc, in_maps, core_ids=list(range(N_CORES)), trace=trace
    )
    last_results = res
    y = np.stack([np.asarray(res.results[i]["out"]).T for i in range(N_CORES)])
    return np.ascontiguousarray(y.astype(np.float32))


# revision 25
# speedup vs baseline: 1.2471x; 1.1879x over previous
"""Depthwise causal conv1d (K=4, dilation=1) on 8 TRN2 NeuronCores.

Reference: x [B=8, T=4096, C=1024] f32, W [4, 1, 1024] f32 (WIO layout),
y[b, t, c] = sum_k W[k, 0, c] * x[b, t - 3 + k, c]  (zero left-pad).

Sharding: pure batch data-parallel — core i computes batch i. On the host we
pre-transpose each batch slice to [C, T+3] (contiguous, causal zero-pad baked
in) so that on-chip the channel dim sits on SBUF partitions (the per-channel
weight becomes a per-partition scalar operand) and the causal time shifts
become free-dim offsets. The device writes y in [C, T] layout; the host
transposes back.

Per-core compute, per channel-group g (8 groups of 128 channels) and time
tile: load x tile [128, tt+3]. Work is split between two fp32-exact paths
to balance engines (VectorE alone would be the bottleneck at ~122us vs the
~94us HBM roofline):
 - DVE path: ScalarE seeds y = x3*W3 (per-partition scale), then 3x
   VectorE scalar_tensor_tensor accumulates the other taps.
 - PE path: per 512-col chunk, 4 accumulating matmuls with diagonal
   [128x128] weight matrices (built on-chip from an identity by ScalarE,
   lazily per group) shift+scale+sum all taps into PSUM; ScalarE evicts.
First/last tiles are tapered small to shrink pipeline fill/drain. Loads go
on the HWDGE ring (nc.sync); stores go on the SWDGE ring (nc.gpsimd) so
compute-gated stores don't head-of-line-block loads — except the last few
tiles' stores, which ride the sync ring once all loads have drained.
"""

from collections import deque

import numpy as np

B, T, C = 8, 4096, 1024
KTAPS = 4
HALO = KTAPS - 1
CG = 128  # channels per partition-group
N_CORES = 8
MM_N = 512  # fp32 moving-operand max free dim / one PSUM bank

# module-level stash so test.py can read profiling info
last_results = None


def _tile_plan(n_groups, t, tt, taper=True):
    """Return [(g, t0, tt_i, on_pe)] covering [0,t) per group.

    Default full-size plan; for the production shape (8 groups, tt=2048)
    use tapered ends and ~20/64 chunks on the PE path.
    """
    if taper and n_groups == 8 and t == 4096 and tt == 2048:
        per_group = {
            0: [(512, False), (512, False), (1024, False), (2048, True)],
            1: [(2048, False), (2048, True)],
            2: [(2048, False), (2048, True)],
            3: [(2048, False), (2048, True)],
            4: [(2048, False), (2048, False)],
            5: [(2048, False), (2048, True)],
            6: [(2048, False), (2048, False)],
            7: [(2048, False), (1024, False), (512, False), (512, False)],
        }
        plan = []
        for g in range(8):
            t0 = 0
            for tt_i, on_pe in per_group[g]:
                plan.append((g, t0, tt_i, on_pe))
                t0 += tt_i
            assert t0 == t
        return plan
    # generic fallback: uniform tiles, every 3rd on PE
    plan = []
    idx = 0
    for g in range(n_groups):
        for j in range(t // tt):
            plan.append((g, j * tt, tt, idx % 3 == 1))
            idx += 1
    return plan


def _end_taper(plan, tt):
    """Split the final (DVE) tile into 1024+512+512 to shorten the drain tail."""
    g, t0, tt_i, on_pe = plan[-1]
    if on_pe or tt_i != tt or tt != 2048:
        return plan
    return plan[:-1] + [
        (g, t0, 1024, False),
        (g, t0 + 1024, 512, False),
        (g, t0 + 1536, 512, False),
    ]


def _build_program(
    c=C,
    t=T,
    tt=2048,
    xbufs=8,
    ybufs=8,
    psbufs=8,
    sync_tail=4,
    taper=True,
    seed_prio=30,
    end_taper=False,
):
    import concourse.bass as bass  # noqa: F401
    import concourse.tile as tile
    from concourse import bacc, mybir

    nc = bacc.Bacc(
        "TRN2",
        target_bir_lowering=False,
        debug=False,
        enable_asserts=False,
        num_devices=N_CORES,
    )
    n_groups = c // CG
    f32 = mybir.dt.float32
    x_ap = nc.dram_tensor("x_t", [c, t + HALO], f32, kind="ExternalInput").ap()
    w_ap = nc.dram_tensor("w_t", [CG, n_groups * KTAPS], f32, kind="ExternalInput").ap()
    eye_ap = nc.dram_tensor("eye", [CG, CG], f32, kind="ExternalInput").ap()
    out_ap = nc.dram_tensor("out", [c, t], f32, kind="ExternalOutput").ap()

    mult = mybir.AluOpType.mult
    add = mybir.AluOpType.add

    plan = _tile_plan(n_groups, t, tt, taper=taper)
    if end_taper:
        plan = _end_taper(plan, tt)
    pe_groups = sorted({g for (g, _, _, on_pe) in plan if on_pe})
    dblk = {g: i * KTAPS * CG for i, g in enumerate(pe_groups)}

    with tile.TileContext(nc) as tc:
        with (
            tc.tile_pool(name="wpool", bufs=1) as wpool,
            tc.tile_pool(name="xpool", bufs=xbufs) as xpool,
            tc.tile_pool(name="ypool", bufs=ybufs) as ypool,
            tc.tile_pool(name="pspool", bufs=psbufs, space="PSUM") as pspool,
        ):
            # tiny dummy ACTIVATE so the ACT function-table load happens
            # during the NEFF preamble instead of on the first seed
            warm = wpool.tile([CG, 1], f32)
            nc.gpsimd.memset(warm[:], 0.0)
            nc.scalar.mul(warm[:], warm[:], 1.0)

            wt = wpool.tile([CG, n_groups * KTAPS], f32)
            nc.sync.dma_start(wt[:], w_ap[:])
            eye = wpool.tile([CG, CG], f32)
            nc.sync.dma_start(eye[:], eye_ap[:])
            wd = None
            if pe_groups:
                wd = wpool.tile([CG, len(pe_groups) * KTAPS * CG], f32)
            diag_built = set()

            tail_stores = deque()  # last tiles' stores, emitted on sync at end
            for ti, (g, t0, tt_i, on_pe) in enumerate(plan):
                r0, r1 = g * CG, (g + 1) * CG
                xt = xpool.tile([CG, tt + HALO], f32, tag="xt")
                xt = xt[:, : tt_i + HALO]
                # x_t is host-padded: column t0 of x_t == time t0 - HALO
                nc.sync.dma_start(xt[:], x_ap[r0:r1, t0 : t0 + tt_i + HALO])
                yt = ypool.tile([CG, tt], f32, tag="yt")
                yt = yt[:, :tt_i]
                if on_pe:
                    if g not in diag_built:
                        # build diag(W[k, g*CG:(g+1)*CG]) on ScalarE, lazily so
                        # early seeds aren't stuck behind all the diag builds
                        diag_built.add(g)
                        for k in range(KTAPS):
                            blk = dblk[g] + k * CG
                            nc.scalar.mul(
                                wd[:, blk : blk + CG],
                                eye[:],
                                wt[:, g * KTAPS + k : g * KTAPS + k + 1],
                            )
                    for c0 in range(0, tt_i, MM_N):
                        ps = pspool.tile([CG, MM_N], f32)
                        for ki, k in enumerate((3, 2, 1, 0)):
                            dcol = dblk[g] + k * CG
                            nc.tensor.matmul(
                                ps[:],
                                wd[:, dcol : dcol + CG],
                                xt[:, c0 + k : c0 + k + MM_N],
                                start=(ki == 0),
                                stop=(ki == KTAPS - 1),
                            )
                        nc.scalar.copy(yt[:, c0 : c0 + MM_N], ps[:])
                else:
                    wcol = g * KTAPS
                    # seed with the last tap on ScalarE (keeps VectorE at 3 ops);
                    # high priority so seeds never queue behind PE-tile
                    # evictions in ScalarE's in-order stream (starves DVE)
                    if seed_prio:
                        with tc.high_priority(offset=seed_prio):
                            nc.scalar.mul(
                                yt[:],
                                xt[:, HALO : HALO + tt_i],
                                wt[:, wcol + 3 : wcol + 4],
                            )
                    else:
                        nc.scalar.mul(
                            yt[:], xt[:, HALO : HALO + tt_i], wt[:, wcol + 3 : wcol + 4]
                        )
                    for k in (2, 1, 0):
                        nc.vector.scalar_tensor_tensor(
                            yt[:],
                            xt[:, k : k + tt_i],
                            wt[:, wcol + k : wcol + k + 1],
                            yt[:],
                            op0=mult,
                            op1=add,
                        )
                if ti < len(plan) - sync_tail:
                    nc.gpsimd.dma_start(out_ap[r0:r1, t0 : t0 + tt_i], yt[:])
                else:
                    tail_stores.append((out_ap[r0:r1, t0 : t0 + tt_i], yt[:]))
            while tail_stores:
                dst, src = tail_stores.popleft()
                nc.sync.dma_start(dst, src)
    nc.compile()
    return nc


def _prep_weights(W: np.ndarray, c=C) -> np.ndarray:
    # wt[p, g*KTAPS + k] = W[k, 0, g*CG + p]
    n_groups = c // CG
    wk = W.reshape(KTAPS, n_groups, CG)  # [k, g, p]
    return np.ascontiguousarray(wk.transpose(2, 1, 0).reshape(CG, n_groups * KTAPS))


def kernel(x: np.ndarray, W: np.ndarray) -> np.ndarray:
    global last_results
    from concourse.bass_utils import run_bass_kernel_spmd

    x = np.asarray(x, dtype=np.float32)
    W = np.asarray(W, dtype=np.float32)
    assert x.shape == (B, T, C) and W.shape == (KTAPS, 1, C)

    nc = _build_program(taper=False, xbufs=6, ybufs=6, sync_tail=0, seed_prio=0)
    wt = _prep_weights(W)
    eye = np.eye(CG, dtype=np.float32)
    zpad = np.zeros((C, HALO), dtype=np.float32)
    in_maps = [
        {
            # [C, T+HALO], causal zero left-pad baked in
            "x_t": np.ascontiguousarray(np.concatenate([zpad, x[i].T], axis=1)),
            "w_t": wt,
            "eye": eye,
        }
        for i in range(N_CORES)
    ]
    import os

    # Only trace when the axon NTFF hook is importable; otherwise force
    # tracing off (a stray BASS_TRACE env var would crash bass_utils).
    trace = False
    if os.environ.get("BASS_TRACE") and not os.environ.get("BASS_NEVER_TRACE"):
        try:
            import antenv.axon_hooks  # noqa: F401

            trace = True
        except ImportError:
            os.environ["BASS_NEVER_TRACE"] = "1"
    res = run_bass_kernel_spmd(
        nc, in_maps, core_ids=list(range(N_CORES)), trace=trace
    )
    last_results = res
    y = np.stack([np.asarray(res.results[i]["out"]).T for i in range(N_CORES)])
    return np.ascontiguousarray(y.astype(np.float32))


# revision 26
# speedup vs baseline: 1.2674x; 1.0163x over previous
"""Depthwise causal conv1d (K=4, dilation=1) on 8 TRN2 NeuronCores.

Reference: x [B=8, T=4096, C=1024] f32, W [4, 1, 1024] f32 (WIO layout),
y[b, t, c] = sum_k W[k, 0, c] * x[b, t - 3 + k, c]  (zero left-pad).

Sharding: pure batch data-parallel — core i computes batch i. On the host we
pre-transpose each batch slice to [C, T+3] (contiguous, causal zero-pad baked
in) so that on-chip the channel dim sits on SBUF partitions (the per-channel
weight becomes a per-partition scalar operand) and the causal time shifts
become free-dim offsets. The device writes y in [C, T] layout; the host
transposes back.

Per-core compute, per channel-group g (8 groups of 128 channels) and time
tile: load x tile [128, tt+3]. Work is split between two fp32-exact paths
to balance engines (VectorE alone would be the bottleneck at ~122us vs the
~94us HBM roofline):
 - DVE path: ScalarE seeds y = x3*W3 (per-partition scale), then 3x
   VectorE scalar_tensor_tensor accumulates the other taps.
 - PE path: per 512-col chunk, 4 accumulating matmuls with diagonal
   [128x128] weight matrices (built on-chip from an identity by ScalarE,
   lazily per group) shift+scale+sum all taps into PSUM; ScalarE evicts.
First/last tiles are tapered small to shrink pipeline fill/drain. Loads go
on the HWDGE ring (nc.sync); stores go on the SWDGE ring (nc.gpsimd) so
compute-gated stores don't head-of-line-block loads — except the last few
tiles' stores, which ride the sync ring once all loads have drained.
"""

from collections import deque

import numpy as np

B, T, C = 8, 4096, 1024
KTAPS = 4
HALO = KTAPS - 1
CG = 128  # channels per partition-group
N_CORES = 8
MM_N = 512  # fp32 moving-operand max free dim / one PSUM bank

# module-level stash so test.py can read profiling info
last_results = None


def _tile_plan(n_groups, t, tt, taper=True):
    """Return [(g, t0, tt_i, on_pe)] covering [0,t) per group.

    Default full-size plan; for the production shape (8 groups, tt=2048)
    use tapered ends and ~20/64 chunks on the PE path.
    """
    if taper and n_groups == 8 and t == 4096 and tt == 2048:
        per_group = {
            0: [(512, False), (512, False), (1024, False), (2048, True)],
            1: [(2048, False), (2048, True)],
            2: [(2048, False), (2048, True)],
            3: [(2048, False), (2048, True)],
            4: [(2048, False), (2048, False)],
            5: [(2048, False), (2048, True)],
            6: [(2048, False), (2048, False)],
            7: [(2048, False), (1024, False), (512, False), (512, False)],
        }
        plan = []
        for g in range(8):
            t0 = 0
            for tt_i, on_pe in per_group[g]:
                plan.append((g, t0, tt_i, on_pe))
                t0 += tt_i
            assert t0 == t
        return plan
    # generic fallback: uniform tiles, every 3rd on PE
    plan = []
    idx = 0
    for g in range(n_groups):
        for j in range(t // tt):
            plan.append((g, j * tt, tt, idx % 3 == 1))
            idx += 1
    return plan


def _end_taper(plan, tt):
    """Split the final (DVE) tile into 1024+512+512 to shorten the drain tail."""
    g, t0, tt_i, on_pe = plan[-1]
    if on_pe or tt_i != tt or tt != 2048:
        return plan
    return plan[:-1] + [
        (g, t0, 1024, False),
        (g, t0 + 1024, 512, False),
        (g, t0 + 1536, 512, False),
    ]


def _build_program(
    c=C,
    t=T,
    tt=2048,
    xbufs=8,
    ybufs=8,
    psbufs=8,
    sync_tail=4,
    taper=True,
    seed_prio=30,
    end_taper=False,
):
    import concourse.bass as bass  # noqa: F401
    import concourse.tile as tile
    from concourse import bacc, mybir

    nc = bacc.Bacc(
        "TRN2",
        target_bir_lowering=False,
        debug=False,
        enable_asserts=False,
        num_devices=N_CORES,
    )
    n_groups = c // CG
    f32 = mybir.dt.float32
    x_ap = nc.dram_tensor("x_t", [c, t + HALO], f32, kind="ExternalInput").ap()
    w_ap = nc.dram_tensor("w_t", [CG, n_groups * KTAPS], f32, kind="ExternalInput").ap()
    eye_ap = nc.dram_tensor("eye", [CG, CG], f32, kind="ExternalInput").ap()
    out_ap = nc.dram_tensor("out", [c, t], f32, kind="ExternalOutput").ap()

    mult = mybir.AluOpType.mult
    add = mybir.AluOpType.add

    plan = _tile_plan(n_groups, t, tt, taper=taper)
    if end_taper:
        plan = _end_taper(plan, tt)
    pe_groups = sorted({g for (g, _, _, on_pe) in plan if on_pe})
    dblk = {g: i * KTAPS * CG for i, g in enumerate(pe_groups)}

    with tile.TileContext(nc) as tc:
        with (
            tc.tile_pool(name="wpool", bufs=1) as wpool,
            tc.tile_pool(name="xpool", bufs=xbufs) as xpool,
            tc.tile_pool(name="ypool", bufs=ybufs) as ypool,
            tc.tile_pool(name="pspool", bufs=psbufs, space="PSUM") as pspool,
        ):
            # tiny dummy ACTIVATE so the ACT function-table load happens
            # during the NEFF preamble instead of on the first seed
            warm = wpool.tile([CG, 1], f32)
            nc.gpsimd.memset(warm[:], 0.0)
            nc.scalar.mul(warm[:], warm[:], 1.0)

            wt = wpool.tile([CG, n_groups * KTAPS], f32)
            nc.sync.dma_start(wt[:], w_ap[:])
            eye = wpool.tile([CG, CG], f32)
            nc.sync.dma_start(eye[:], eye_ap[:])
            wd = None
            if pe_groups:
                wd = wpool.tile([CG, len(pe_groups) * KTAPS * CG], f32)
            diag_built = set()

            tail_stores = deque()  # last tiles' stores, emitted on sync at end
            for ti, (g, t0, tt_i, on_pe) in enumerate(plan):
                r0, r1 = g * CG, (g + 1) * CG
                xt = xpool.tile([CG, tt + HALO], f32, tag="xt")
                xt = xt[:, : tt_i + HALO]
                # x_t is host-padded: column t0 of x_t == time t0 - HALO
                nc.sync.dma_start(xt[:], x_ap[r0:r1, t0 : t0 + tt_i + HALO])
                yt = ypool.tile([CG, tt], f32, tag="yt")
                yt = yt[:, :tt_i]
                if on_pe:
                    if g not in diag_built:
                        # build diag(W[k, g*CG:(g+1)*CG]) on ScalarE, lazily so
                        # early seeds aren't stuck behind all the diag builds
                        diag_built.add(g)
                        for k in range(KTAPS):
                            blk = dblk[g] + k * CG
                            nc.scalar.mul(
                                wd[:, blk : blk + CG],
                                eye[:],
                                wt[:, g * KTAPS + k : g * KTAPS + k + 1],
                            )
                    for c0 in range(0, tt_i, MM_N):
                        ps = pspool.tile([CG, MM_N], f32)
                        for ki, k in enumerate((3, 2, 1, 0)):
                            dcol = dblk[g] + k * CG
                            nc.tensor.matmul(
                                ps[:],
                                wd[:, dcol : dcol + CG],
                                xt[:, c0 + k : c0 + k + MM_N],
                                start=(ki == 0),
                                stop=(ki == KTAPS - 1),
                            )
                        nc.scalar.copy(yt[:, c0 : c0 + MM_N], ps[:])
                else:
                    wcol = g * KTAPS
                    # seed with the last tap on ScalarE (keeps VectorE at 3 ops);
                    # high priority so seeds never queue behind PE-tile
                    # evictions in ScalarE's in-order stream (starves DVE)
                    if seed_prio:
                        with tc.high_priority(offset=seed_prio):
                            nc.scalar.mul(
                                yt[:],
                                xt[:, HALO : HALO + tt_i],
                                wt[:, wcol + 3 : wcol + 4],
                            )
                    else:
                        nc.scalar.mul(
                            yt[:], xt[:, HALO : HALO + tt_i], wt[:, wcol + 3 : wcol + 4]
                        )
                    for k in (2, 1, 0):
                        nc.vector.scalar_tensor_tensor(
                            yt[:],
                            xt[:, k : k + tt_i],
                            wt[:, wcol + k : wcol + k + 1],
                            yt[:],
                            op0=mult,
                            op1=add,
                        )
                if ti < len(plan) - sync_tail:
                    nc.gpsimd.dma_start(out_ap[r0:r1, t0 : t0 + tt_i], yt[:])
                else:
                    tail_stores.append((out_ap[r0:r1, t0 : t0 + tt_i], yt[:]))
            while tail_stores:
                dst, src = tail_stores.popleft()
                nc.sync.dma_start(dst, src)
    nc.compile()
    return nc


def _prep_weights(W: np.ndarray, c=C) -> np.ndarray:
    # wt[p, g*KTAPS + k] = W[k, 0, g*CG + p]
    n_groups = c // CG
    wk = W.reshape(KTAPS, n_groups, CG)  # [k, g, p]
    return np.ascontiguousarray(wk.transpose(2, 1, 0).reshape(CG, n_groups * KTAPS))


def kernel(x: np.ndarray, W: np.ndarray) -> np.ndarray:
    global last_results
    from concourse.bass_utils import run_bass_kernel_spmd

    x = np.asarray(x, dtype=np.float32)
    W = np.asarray(W, dtype=np.float32)
    assert x.shape == (B, T, C) and W.shape == (KTAPS, 1, C)

    nc = _build_program(
        taper=False, xbufs=6, ybufs=6, sync_tail=0, seed_prio=0, end_taper=True
    )
    wt = _prep_weights(W)
    eye = np.eye(CG, dtype=np.float32)
    zpad = np.zeros((C, HALO), dtype=np.float32)
    in_maps = [
        {
            # [C, T+HALO], causal zero left-pad baked in
            "x_t": np.ascontiguousarray(np.concatenate([zpad, x[i].T], axis=1)),
            "w_t": wt,
            "eye": eye,
        }
        for i in range(N_CORES)
    ]
    import os

    # Only trace when the axon NTFF hook is importable; otherwise force
    # tracing off (a stray BASS_TRACE env var would crash bass_utils).
    trace = False
    if os.environ.get("BASS_TRACE") and not os.environ.get("BASS_NEVER_TRACE"):
        try:
            import antenv.axon_hooks  # noqa: F401

            trace = True
        except ImportError:
            os.environ["BASS_NEVER_TRACE"] = "1"
    res = run_bass_kernel_spmd(
        nc, in_maps, core_ids=list(range(N_CORES)), trace=trace
    )
    last_results = res
    y = np.stack([np.asarray(res.results[i]["out"]).T for i in range(N_CORES)])
    return np.ascontiguousarray(y.astype(np.float32))
